# Initial kernel scaffold
#
"""Contextual loss kernel for Trainium2 (8 NeuronCores, SPMD over batch).

Math (per sample n):
    cos[p,q] = <x_n[:,p], y_n[:,q]>          (channel-normalized, centered)
    cx[p,q]  = softmax_q(beta_p * cos[p,q]),  beta_p = 2 / (1 - max_q cos[p,q] + EPS)
    loss_n   = -log(mean_q max_p cx[p,q] + EPS)
    out      = mean_n loss_n

Each core handles one sample (N=8); the device produces the row-block
max-accumulated cx matrix acc[128, 2304] (max over the 18 query row blocks,
bf16); the host finishes with max over the 128 partitions, mean over q, log,
and the batch mean.

v6 structure (driven by trace analysis):
  - ONE psum pool of eight single-bank [128,512] tiles; each block uses 5
    (4x512 + 256).  Reuse distance is 8 allocs = 1.6 blocks, so block i+1's
    matmuls are gated only by block i's first two exps -> the in-order PE
    queue never starves and the PE stays at full pstate.
  - inputs shipped bf16 (half DMA); all SBUF data bf16 except PSUM, e, and
    per-row stats.
  - norm: centers on DVE TS (packed), squares via DVE TT for y (head
    critical) / ACT Square for x, ACT Abs_reciprocal_sqrt for 1/|v|.
  - per block: DVE psum row-max reduces + tiny combines; GpSimd
    normalize_recip for beta AND for the whole es = e/rowsum pass; ACT Exp
    (accum rowsum) + Copy rowsum-combine; DVE packed TT max-accumulate.
  - block i's scale pass is emitted at the TOP of iteration i+1 (Pool queue:
    before beta(i+1)); its TT max-accumulate after halfd(i+1) on DVE.
"""

import ml_dtypes
import numpy as np

import concourse.bacc as bacc
import concourse.mybir as mybir
import concourse.tile as tile
from concourse import library_config
from concourse.bass_utils import run_bass_kernel_spmd

N, C, H, W = 8, 512, 48, 48
HW = H * W  # 2304
KC = C // 128  # 4 channel chunks
NBLK = HW // 128  # 18 row blocks
PANELS = [(0, 512), (512, 512), (1024, 512), (1536, 512), (2048, 256)]
NP = len(PANELS)
TT_PANELS = [(0, 2048), (2048, 256)]  # acc max-accumulate slices
EPS = 1e-5

F32 = mybir.dt.float32
BF16 = mybir.dt.bfloat16
AF = mybir.ActivationFunctionType
OP = mybir.AluOpType
AX = mybir.AxisListType


def build_bass():
    nc = bacc.Bacc("TRN2", target_bir_lowering=False, debug=False)
    pred_d = nc.dram_tensor("pred", (C, HW), BF16, kind="ExternalInput")
    targ_d = nc.dram_tensor("target", (C, HW), BF16, kind="ExternalInput")
    negmu_d = nc.dram_tensor("negmu", (128, KC), F32, kind="ExternalInput")
    acc_d = nc.dram_tensor("acc_out", (128, HW), BF16, kind="ExternalOutput")

    with tile.TileContext(nc) as tc:
        from contextlib import ExitStack
        with ExitStack() as ctx:
            singles = ctx.enter_context(tc.tile_pool(name="singles", bufs=1))
            xn_pool = ctx.enter_context(tc.tile_pool(name="xn", bufs=2 * KC))
            psum = ctx.enter_context(
                tc.tile_pool(name="ps", bufs=8, space="PSUM"))

            negmu_sb = singles.tile([128, KC], F32)
            nc.sync.dma_start(out=negmu_sb, in_=negmu_d[:, :])
            ones_f32 = singles.tile([128, 128], F32)
            nc.vector.memset(ones_f32, 1.0)
            ones_sb = singles.tile([128, 128], BF16)
            nc.vector.tensor_copy(ones_sb, ones_f32)
            one_col = singles.tile([128, 1], F32)
            nc.vector.memset(one_col, 1.0)
            acc = singles.tile([128, HW], BF16)
            nc.gpsimd.memset(acc, 0.0)

            xn = [xn_pool.tile([128, HW], BF16, name=f"xn_{k}", tag=f"xn_{k}",
                               bufs=1) for k in range(KC)]
            yn = [xn_pool.tile([128, HW], BF16, name=f"yn_{k}", tag=f"yn_{k}",
                               bufs=1) for k in range(KC)]

            # ---------------- normalization ----------------
            with ExitStack() as nctx:
                raw_y = nctx.enter_context(tc.tile_pool(name="raw_y", bufs=KC))
                raw_x = nctx.enter_context(tc.tile_pool(name="raw_x", bufs=8))
                sq_pool = nctx.enter_context(tc.tile_pool(name="sq", bufs=4))
                xc_pool = nctx.enter_context(tc.tile_pool(name="xc", bufs=8))
                r_pool = nctx.enter_context(tc.tile_pool(name="r", bufs=2))

                yraw = []
                for k in range(KC):
                    t = raw_y.tile([128, HW], BF16, name=f"yraw_{k}", tag="yraw")
                    nc.sync.dma_start(out=t, in_=targ_d[k * 128:(k + 1) * 128, :])
                    yraw.append(t)
                NORM_PANELS = [(0, 1024), (1024, 1024), (2048, 256)]
                xraw = {}
                for (off, w) in NORM_PANELS:
                    for k in range(KC):
                        t = raw_x.tile([128, 1024], BF16, name=f"xr_{off}_{k}",
                                       tag="xraw")
                        nc.sync.dma_start(
                            out=t[:, :w],
                            in_=pred_d[k * 128:(k + 1) * 128, off:off + w])
                        xraw[(off, k)] = t

                # normalize_recip lives in the attn ucode library; emitted
                # after the input DMAs so the IRAM load doesn't delay them.
                nc.gpsimd.load_library(library_config.attn)

                def norm_panel(pfx, off, w, raw_of_k, r, out_of_k):
                    xcs = []
                    for k in range(KC):
                        xc = xc_pool.tile([128, 1024], BF16,
                                          name=f"c{pfx}_{off}_{k}", tag="xc")
                        nc.vector.tensor_scalar(
                            out=xc[:, :w], in0=raw_of_k(k),
                            scalar1=negmu_sb[:, k:k + 1], scalar2=None,
                            op0=OP.add)
                        xcs.append(xc)
                    sqs = []
                    for k in range(KC):
                        t = sq_pool.tile([128, 1024], BF16,
                                         name=f"t{pfx}_{off}_{k}", tag="sq")
                        nc.scalar.activation(out=t[:, :w], in_=raw_of_k(k),
                                             func=AF.Square,
                                             bias=negmu_sb[:, k:k + 1],
                                             scale=1.0)
                        sqs.append(t)
                    # norm^2 matmuls + rsqrt in single-bank 512 pieces so the
                    # one 8x512 psum pool serves norm and main loop alike
                    for s in range(0, w, 512):
                        e_ = min(w, s + 512)
                        psn = psum.tile([128, 512], F32,
                                        name=f"psn{pfx}_{off}_{s}", tag="ps")
                        for k in range(KC):
                            nc.tensor.matmul(psn[:, :e_ - s], ones_sb,
                                             sqs[k][:, s:e_],
                                             start=(k == 0), stop=(k == KC - 1))
                        nc.scalar.activation(out=r[:, off + s:off + e_],
                                             in_=psn[:, :e_ - s],
                                             func=AF.Abs_reciprocal_sqrt,
                                             scale=1.0)
                    for k in range(KC):
                        nc.vector.tensor_tensor(
                            out=out_of_k(k)[:, off:off + w], in0=xcs[k][:, :w],
                            in1=r[:, off:off + w], op=OP.mult)

                ry = r_pool.tile([128, HW], BF16, name="ry", tag="r")
                for (off, w) in NORM_PANELS:
                    norm_panel("y", off, w,
                               lambda k, off=off, w=w: yraw[k][:, off:off + w],
                               ry, lambda k: yn[k])
                rx = r_pool.tile([128, HW], BF16, name="rx", tag="r")
                for (off, w) in NORM_PANELS:
                    norm_panel("x", off, w,
                               lambda k, off=off, w=w: xraw[(off, k)][:, :w],
                               rx, lambda k: xn[k])

            # ---------------- main loop ----------------
            e_pool = ctx.enter_context(tc.tile_pool(name="e", bufs=3))
            es_pool = ctx.enter_context(tc.tile_pool(name="es", bufs=3))
            st_pool = ctx.enter_context(tc.tile_pool(name="stats", bufs=4))

            pending = None  # (e, rstot, idx) of the previous block

            def flush_scale():
                # es = e / rowsum in ONE GpSimd pass; sits on the Pool queue
                # BEFORE beta(i) so it runs during this block's matmuls.
                pe, prstot, pi = pending
                es = es_pool.tile([128, HW], BF16, name=f"es_{pi}", tag="es")
                nc.gpsimd.normalize_recip(out_ap=es, in_ap=pe, denom_ap=prstot)
                return es

            def flush_ttmax(es):
                # packed TT max-accumulate, emitted after halfd so it stays
                # out of the reduce->beta critical chain on the DVE queue.
                for (off, w) in TT_PANELS:
                    nc.vector.tensor_tensor(
                        out=acc[:, off:off + w], in0=es[:, off:off + w],
                        in1=acc[:, off:off + w], op=OP.max)

            for i in range(NBLK):
                es_prev = flush_scale() if pending is not None else None

                rows = slice(i * 128, (i + 1) * 128)
                mx = st_pool.tile([128, NP], F32, name=f"mx_{i}", tag="mx")
                psc = []
                for j, (off, w) in enumerate(PANELS):
                    ps = psum.tile([128, 512], F32, name=f"ps_{i}_{j}", tag="ps")
                    for k in range(KC):
                        nc.tensor.matmul(ps[:, :w], xn[k][:, rows],
                                         yn[k][:, off:off + w],
                                         start=(k == 0), stop=(k == KC - 1))
                    nc.vector.reduce_max(mx[:, j:j + 1], ps[:, :w], axis=AX.X)
                    psc.append(ps)

                m = st_pool.tile([128, 1], F32, name=f"m_{i}", tag="m")
                halfd = st_pool.tile([128, 1], F32, name=f"halfd_{i}", tag="halfd")
                beta = st_pool.tile([128, 1], F32, name=f"beta_{i}", tag="beta")
                nc.vector.reduce_max(m, mx, axis=AX.X)
                # halfd = 0.5*(1+EPS) - 0.5*m ; beta = 1/halfd = 2/(1 - m + EPS)
                nc.vector.tensor_scalar(out=halfd, in0=m, scalar1=-0.5,
                                        scalar2=0.5 * (1.0 + EPS),
                                        op0=OP.mult, op1=OP.add)
                nc.gpsimd.normalize_recip(out_ap=beta, in_ap=one_col,
                                          denom_ap=halfd)

                if es_prev is not None:
                    flush_ttmax(es_prev)
                    pending = None

                e = e_pool.tile([128, HW], F32, name=f"e_{i}", tag="e")
                rs = st_pool.tile([128, NP], F32, name=f"rs_{i}", tag="rs")
                for j, (off, w) in enumerate(PANELS):
                    nc.scalar.activation(out=e[:, off:off + w],
                                         in_=psc[j][:, :w],
                                         func=AF.Exp, scale=beta[:, 0:1],
                                         accum_out=rs[:, j:j + 1])
                # rowsum combine on ACT (Copy + free-dim accumulator)
                rstot = st_pool.tile([128, 1], F32, name=f"rst_{i}", tag="rst")
                rsc = st_pool.tile([128, NP], F32, name=f"rsc_{i}", tag="rsc")
                nc.scalar.activation(out=rsc, in_=rs, func=AF.Copy,
                                     accum_out=rstot)
                pending = (e, rstot, i)

            es_last = flush_scale()
            flush_ttmax(es_last)

            # ship acc
            for (off, w) in TT_PANELS:
                nc.sync.dma_start(out=acc_d[:, off:off + w],
                                  in_=acc[:, off:off + w])

    nc.compile()
    return nc


_NC_CACHE = None


def _get_nc():
    global _NC_CACHE
    if _NC_CACHE is None:
        _NC_CACHE = build_bass()
    return _NC_CACHE


def make_in_maps(pred: np.ndarray, target: np.ndarray):
    y_mu = target.reshape(N, C, HW).astype(np.float64).mean(axis=(0, 2))
    negmu = np.ascontiguousarray((-y_mu).astype(np.float32).reshape(KC, 128).T)
    pred16 = pred.reshape(N, C, HW).astype(ml_dtypes.bfloat16)
    targ16 = target.reshape(N, C, HW).astype(ml_dtypes.bfloat16)
    return [{
        "pred": np.ascontiguousarray(pred16[n]),
        "target": np.ascontiguousarray(targ16[n]),
        "negmu": negmu,
    } for n in range(N)]


def kernel(pred: np.ndarray, target: np.ndarray) -> np.ndarray:
    pred = np.asarray(pred, dtype=np.float32)
    target = np.asarray(target, dtype=np.float32)
    assert pred.shape == (N, C, H, W) and target.shape == (N, C, H, W)

    nc = _get_nc()
    res = run_bass_kernel_spmd(nc, make_in_maps(pred, target),
                               core_ids=list(range(N)))

    losses = np.empty(N, dtype=np.float64)
    for n in range(N):
        acc = np.asarray(res.results[n]["acc_out"]).astype(np.float64)
        colmax = acc.max(axis=0)  # max over query rows
        cx_n = colmax.mean()  # mean over keys
        losses[n] = -np.log(cx_n + EPS)
    return np.float32(losses.mean())



# revision 1
# speedup vs baseline: 1.1628x; 1.1628x over previous
"""Contextual loss kernel for Trainium2 (8 NeuronCores, SPMD over batch).

Math (per sample n):
    cos[p,q] = <x_n[:,p], y_n[:,q]>          (channel-normalized, centered)
    cx[p,q]  = softmax_q(beta_p * cos[p,q]),  beta_p = 2 / (1 - max_q cos[p,q] + EPS)
    loss_n   = -log(mean_q max_p cx[p,q] + EPS)
    out      = mean_n loss_n

Each core handles one sample (N=8); the device produces the row-block
max-accumulated cx matrix acc[128, 2304] (max over the 18 query row blocks,
bf16); the host finishes with max over the 128 partitions, mean over q, log,
and the batch mean.

v6 structure (driven by trace analysis):
  - ONE psum pool of eight single-bank [128,512] tiles; each block uses 5
    (4x512 + 256).  Reuse distance is 8 allocs = 1.6 blocks, so block i+1's
    matmuls are gated only by block i's first two exps -> the in-order PE
    queue never starves and the PE stays at full pstate.
  - inputs shipped bf16 (half DMA); all SBUF data bf16 except PSUM, e, and
    per-row stats.
  - norm: centers on DVE TS (packed), squares via DVE TT for y (head
    critical) / ACT Square for x, ACT Abs_reciprocal_sqrt for 1/|v|.
  - per block: DVE psum row-max reduces + tiny combines; GpSimd
    normalize_recip for beta AND for the whole es = e/rowsum pass; ACT Exp
    (accum rowsum) + Copy rowsum-combine; DVE packed TT max-accumulate.
  - block i's scale pass is emitted at the TOP of iteration i+1 (Pool queue:
    before beta(i+1)); its TT max-accumulate after halfd(i+1) on DVE.
"""

import ml_dtypes
import numpy as np

import concourse.bacc as bacc
import concourse.mybir as mybir
import concourse.tile as tile
from concourse import library_config
from concourse.bass_utils import run_bass_kernel_spmd

N, C, H, W = 8, 512, 48, 48
HW = H * W  # 2304
KC = C // 128  # 4 channel chunks
NBLK = HW // 128  # 18 row blocks
PANELS = [(0, 512), (512, 512), (1024, 512), (1536, 512), (2048, 256)]
NP = len(PANELS)
TT_PANELS = [(0, 2048), (2048, 256)]  # acc max-accumulate slices
EPS = 1e-5

F32 = mybir.dt.float32
BF16 = mybir.dt.bfloat16
AF = mybir.ActivationFunctionType
OP = mybir.AluOpType
AX = mybir.AxisListType


def build_bass():
    nc = bacc.Bacc("TRN2", target_bir_lowering=False, debug=False)
    pred_d = nc.dram_tensor("pred", (C, HW), BF16, kind="ExternalInput")
    targ_d = nc.dram_tensor("target", (C, HW), BF16, kind="ExternalInput")
    negmu_d = nc.dram_tensor("negmu", (128, KC), F32, kind="ExternalInput")
    acc_d = nc.dram_tensor("acc_out", (128, HW), BF16, kind="ExternalOutput")

    with tile.TileContext(nc) as tc:
        from contextlib import ExitStack
        with ExitStack() as ctx:
            singles = ctx.enter_context(tc.tile_pool(name="singles", bufs=1))
            xn_pool = ctx.enter_context(tc.tile_pool(name="xn", bufs=2 * KC))
            psum = ctx.enter_context(
                tc.tile_pool(name="ps", bufs=8, space="PSUM"))

            negmu_sb = singles.tile([128, KC], F32)
            nc.sync.dma_start(out=negmu_sb, in_=negmu_d[:, :])
            ones_f32 = singles.tile([128, 128], F32)
            nc.vector.memset(ones_f32, 1.0)
            ones_sb = singles.tile([128, 128], BF16)
            nc.vector.tensor_copy(ones_sb, ones_f32)
            one_col = singles.tile([128, 1], F32)
            nc.vector.memset(one_col, 1.0)
            acc = singles.tile([128, HW], BF16)
            nc.gpsimd.memset(acc, 0.0)

            xn = [xn_pool.tile([128, HW], BF16, name=f"xn_{k}", tag=f"xn_{k}",
                               bufs=1) for k in range(KC)]
            yn = [xn_pool.tile([128, HW], BF16, name=f"yn_{k}", tag=f"yn_{k}",
                               bufs=1) for k in range(KC)]

            # ---------------- normalization ----------------
            with ExitStack() as nctx:
                raw_y = nctx.enter_context(tc.tile_pool(name="raw_y", bufs=KC))
                raw_x = nctx.enter_context(tc.tile_pool(name="raw_x", bufs=8))
                sq_pool = nctx.enter_context(tc.tile_pool(name="sq", bufs=4))
                xc_pool = nctx.enter_context(tc.tile_pool(name="xc", bufs=8))
                r_pool = nctx.enter_context(tc.tile_pool(name="r", bufs=2))

                yraw = []
                for k in range(KC):
                    t = raw_y.tile([128, HW], BF16, name=f"yraw_{k}", tag="yraw")
                    nc.sync.dma_start(out=t, in_=targ_d[k * 128:(k + 1) * 128, :])
                    yraw.append(t)
                NORM_PANELS = [(0, 1024), (1024, 1024), (2048, 256)]
                xraw = {}
                for (off, w) in NORM_PANELS:
                    for k in range(KC):
                        t = raw_x.tile([128, 1024], BF16, name=f"xr_{off}_{k}",
                                       tag="xraw")
                        nc.sync.dma_start(
                            out=t[:, :w],
                            in_=pred_d[k * 128:(k + 1) * 128, off:off + w])
                        xraw[(off, k)] = t

                # normalize_recip lives in the attn ucode library; emitted
                # after the input DMAs so the IRAM load doesn't delay them.
                nc.gpsimd.load_library(library_config.attn)

                def norm_panel(pfx, off, w, raw_of_k, r, out_of_k):
                    xcs = []
                    for k in range(KC):
                        xc = xc_pool.tile([128, 1024], BF16,
                                          name=f"c{pfx}_{off}_{k}", tag="xc")
                        nc.vector.tensor_scalar(
                            out=xc[:, :w], in0=raw_of_k(k),
                            scalar1=negmu_sb[:, k:k + 1], scalar2=None,
                            op0=OP.add)
                        xcs.append(xc)
                    sqs = []
                    for k in range(KC):
                        t = sq_pool.tile([128, 1024], BF16,
                                         name=f"t{pfx}_{off}_{k}", tag="sq")
                        nc.scalar.activation(out=t[:, :w], in_=raw_of_k(k),
                                             func=AF.Square,
                                             bias=negmu_sb[:, k:k + 1],
                                             scale=1.0)
                        sqs.append(t)
                    # norm^2 matmuls + rsqrt in single-bank 512 pieces so the
                    # one 8x512 psum pool serves norm and main loop alike
                    for s in range(0, w, 512):
                        e_ = min(w, s + 512)
                        psn = psum.tile([128, 512], F32,
                                        name=f"psn{pfx}_{off}_{s}", tag="ps")
                        for k in range(KC):
                            nc.tensor.matmul(psn[:, :e_ - s], ones_sb,
                                             sqs[k][:, s:e_],
                                             start=(k == 0), stop=(k == KC - 1))
                        nc.scalar.activation(out=r[:, off + s:off + e_],
                                             in_=psn[:, :e_ - s],
                                             func=AF.Abs_reciprocal_sqrt,
                                             scale=1.0)
                    for k in range(KC):
                        nc.vector.tensor_tensor(
                            out=out_of_k(k)[:, off:off + w], in0=xcs[k][:, :w],
                            in1=r[:, off:off + w], op=OP.mult)

                ry = r_pool.tile([128, HW], BF16, name="ry", tag="r")
                for (off, w) in NORM_PANELS:
                    norm_panel("y", off, w,
                               lambda k, off=off, w=w: yraw[k][:, off:off + w],
                               ry, lambda k: yn[k])
                rx = r_pool.tile([128, HW], BF16, name="rx", tag="r")
                for (off, w) in NORM_PANELS:
                    norm_panel("x", off, w,
                               lambda k, off=off, w=w: xraw[(off, k)][:, :w],
                               rx, lambda k: xn[k])

            # ---------------- main loop ----------------
            e_pool = ctx.enter_context(tc.tile_pool(name="e", bufs=3))
            es_pool = ctx.enter_context(tc.tile_pool(name="es", bufs=3))
            st_pool = ctx.enter_context(tc.tile_pool(name="stats", bufs=4))

            pending = None  # (e, rstot, idx) of the previous block

            def flush_scale():
                # es = e / rowsum in ONE GpSimd pass; sits on the Pool queue
                # BEFORE beta(i) so it runs during this block's matmuls.
                pe, prstot, pi = pending
                es = es_pool.tile([128, HW], BF16, name=f"es_{pi}", tag="es")
                nc.gpsimd.normalize_recip(out_ap=es, in_ap=pe, denom_ap=prstot)
                return es

            def flush_ttmax(es):
                # packed TT max-accumulate, emitted after halfd so it stays
                # out of the reduce->beta critical chain on the DVE queue.
                for (off, w) in TT_PANELS:
                    nc.vector.tensor_tensor(
                        out=acc[:, off:off + w], in0=es[:, off:off + w],
                        in1=acc[:, off:off + w], op=OP.max)

            for i in range(NBLK):
                es_prev = flush_scale() if pending is not None else None

                rows = slice(i * 128, (i + 1) * 128)
                mx = st_pool.tile([128, NP], F32, name=f"mx_{i}", tag="mx")
                psc = []
                for j, (off, w) in enumerate(PANELS):
                    ps = psum.tile([128, 512], F32, name=f"ps_{i}_{j}", tag="ps")
                    for k in range(KC):
                        nc.tensor.matmul(ps[:, :w], xn[k][:, rows],
                                         yn[k][:, off:off + w],
                                         start=(k == 0), stop=(k == KC - 1))
                    nc.vector.reduce_max(mx[:, j:j + 1], ps[:, :w], axis=AX.X)
                    psc.append(ps)

                m = st_pool.tile([128, 1], F32, name=f"m_{i}", tag="m")
                halfd = st_pool.tile([128, 1], F32, name=f"halfd_{i}", tag="halfd")
                beta = st_pool.tile([128, 1], F32, name=f"beta_{i}", tag="beta")
                nc.vector.reduce_max(m, mx, axis=AX.X)
                # halfd = 0.5*(1+EPS) - 0.5*m ; beta = 1/halfd = 2/(1 - m + EPS)
                nc.vector.tensor_scalar(out=halfd, in0=m, scalar1=-0.5,
                                        scalar2=0.5 * (1.0 + EPS),
                                        op0=OP.mult, op1=OP.add)
                nc.gpsimd.normalize_recip(out_ap=beta, in_ap=one_col,
                                          denom_ap=halfd)

                if es_prev is not None:
                    flush_ttmax(es_prev)
                    pending = None

                e = e_pool.tile([128, HW], F32, name=f"e_{i}", tag="e")
                rs = st_pool.tile([128, NP], F32, name=f"rs_{i}", tag="rs")
                for j, (off, w) in enumerate(PANELS):
                    nc.scalar.activation(out=e[:, off:off + w],
                                         in_=psc[j][:, :w],
                                         func=AF.Exp, scale=beta[:, 0:1],
                                         accum_out=rs[:, j:j + 1])
                # rowsum combine on ACT (Copy + free-dim accumulator)
                rstot = st_pool.tile([128, 1], F32, name=f"rst_{i}", tag="rst")
                rsc = st_pool.tile([128, NP], F32, name=f"rsc_{i}", tag="rsc")
                nc.scalar.activation(out=rsc, in_=rs, func=AF.Copy,
                                     accum_out=rstot)
                pending = (e, rstot, i)

            es_last = flush_scale()
            flush_ttmax(es_last)

            # ship acc
            for (off, w) in TT_PANELS:
                nc.sync.dma_start(out=acc_d[:, off:off + w],
                                  in_=acc[:, off:off + w])

    nc.compile()
    return nc


_NC_CACHE = None


def _get_nc():
    global _NC_CACHE
    if _NC_CACHE is None:
        _NC_CACHE = build_bass()
    return _NC_CACHE


def make_in_maps(pred: np.ndarray, target: np.ndarray):
    y_mu = target.reshape(N, C, HW).astype(np.float64).mean(axis=(0, 2))
    negmu = np.ascontiguousarray((-y_mu).astype(np.float32).reshape(KC, 128).T)
    pred16 = pred.reshape(N, C, HW).astype(ml_dtypes.bfloat16)
    targ16 = target.reshape(N, C, HW).astype(ml_dtypes.bfloat16)
    return [{
        "pred": np.ascontiguousarray(pred16[n]),
        "target": np.ascontiguousarray(targ16[n]),
        "negmu": negmu,
    } for n in range(N)]


def kernel(pred: np.ndarray, target: np.ndarray) -> np.ndarray:
    pred = np.asarray(pred, dtype=np.float32)
    target = np.asarray(target, dtype=np.float32)
    assert pred.shape == (N, C, H, W) and target.shape == (N, C, H, W)

    nc = _get_nc()
    res = run_bass_kernel_spmd(nc, make_in_maps(pred, target),
                               core_ids=list(range(N)))

    losses = np.empty(N, dtype=np.float64)
    for n in range(N):
        acc = np.asarray(res.results[n]["acc_out"]).astype(np.float64)
        colmax = acc.max(axis=0)  # max over query rows
        cx_n = colmax.mean()  # mean over keys
        losses[n] = -np.log(cx_n + EPS)
    return np.float32(losses.mean())

